# revision 4
# baseline (speedup 1.0000x reference)
"""Trainium2 Bass kernel for nn_Damping_layer: out = kipf_term - lbda[:, None] * input_term.

Sharding: pure row-parallel over the n_nodes axis across 8 NeuronCores
(12500 rows per core), no cross-core communication. Each core's shard is
host-padded to 12544 rows so it divides into 7 uniform tiles of
[128 partitions x 14 rows/partition].

The kernel is HBM-bandwidth bound, so all DRAM I/O is fp16: the host
casts input_term/kipf_term to fp16 (norm relative error ~3e-4, far
inside the 2e-2 gate) and upcasts the fp16 output back to f32. This
halves DRAM traffic vs f32 (19.3 MB/core vs 38.5 MB/core).

input_term and kipf_term are interleaved on host into one DRAM tensor z
laid out [tile, partition, half, row, col], so a whole-tile load is a
single dma_start whose per-partition run is 14336 B contiguous.

Ring discipline: the SDMA engines round-robin between their queues
per-DESCRIPTOR, so the instantaneous byte split between the two HWDGE
rings equals the ratio of their descriptor sizes. Loads:stores is 2:1
in bytes and load:store descriptors are 14336 B : 7168 B = 2:1, so
putting ALL loads on the SP ring and ALL stores on the ACT ring makes
the round-robin hand each stream exactly its demand share — no ring
ever starves and no store can head-of-line-block a load.

lbda is negated and pre-shuffled on host into the matching
[partition, group] layout so the fused DVE op
    out = (input * (-lbda)) + kipf            (InstTensorScalarPtr)
consumes it directly as a per-partition scalar, one op per 128-row
group. The first/last tiles are emitted as small j-sub-chunks so the
pipeline ramps in (DVE starts early) and drains out quickly.
"""

import numpy as np

N_NODES = 100000
N_FEAT = 256
N_CORES = 8
ROWS_PER_CORE = N_NODES // N_CORES  # 12500

R_PP = 14                       # rows per partition in a tile
TILE_ROWS = 128 * R_PP          # 1792 rows per tile
N_TILES = 7                     # tiles per core
PAD_ROWS = N_TILES * TILE_ROWS  # 12544 rows per core after padding
LB_COLS = N_TILES * R_PP        # 98
ZBUFS = 10                      # loads emitted ahead of compute
OBUFS = 4

_CACHE = {}


def _build_nc():
    from contextlib import ExitStack

    import concourse.bacc as bacc
    import concourse.mybir as mybir
    import concourse.tile as tile

    FP32 = mybir.dt.float32
    FP16 = mybir.dt.float16
    nc = bacc.Bacc(
        "TRN2", target_bir_lowering=False, debug=False, num_devices=N_CORES
    )
    z = nc.dram_tensor(
        "z", [2 * PAD_ROWS, N_FEAT], FP16, kind="ExternalInput"
    ).ap()
    lb = nc.dram_tensor("lb", [128, LB_COLS], FP32, kind="ExternalInput").ap()
    o = nc.dram_tensor("o", [PAD_ROWS, N_FEAT], FP16, kind="ExternalOutput").ap()

    # z layout (host-built): [t, p, h, j, c] with h=0 input rows, h=1 kipf
    # rows; partition p holds one 2*R_PP*512 B contiguous DRAM run per tile.
    zv = z.rearrange(
        "(t p h j) c -> t p (h j c)", t=N_TILES, h=2, p=128, j=R_PP
    )
    ov = o.rearrange("(t p j) c -> t p (j c)", t=N_TILES, p=128, j=R_PP)

    MULT = mybir.AluOpType.mult
    ADD = mybir.AluOpType.add
    KOFF = R_PP * N_FEAT  # kipf half offset within a z tile

    with tile.TileContext(nc) as tc, ExitStack() as ctx:
        const = ctx.enter_context(tc.tile_pool(name="const", bufs=1))
        zpool = ctx.enter_context(tc.tile_pool(name="zp", bufs=ZBUFS))
        opool = ctx.enter_context(tc.tile_pool(name="op", bufs=OBUFS))

        # nlb (host-negated lbda) rides SWDGE (gpsimd), keeping both HWDGE
        # rings' heads free for the data stream.
        nlb = const.tile([128, LB_COLS], FP32)
        nc.gpsimd.dma_start(out=nlb[:], in_=lb[:])

        # Work list: first/last tiles in small sub-chunks so the pipeline
        # ramps in and drains out quickly; full tiles in between.
        chunks = [(0, 0, 4), (0, 4, 9), (0, 9, 14)]
        chunks += [(t, 0, R_PP) for t in range(1, N_TILES - 1)]
        chunks += [
            (N_TILES - 1, 0, 5),
            (N_TILES - 1, 5, 10),
            (N_TILES - 1, 10, 12),
            (N_TILES - 1, 12, 14),
        ]

        def emit_load(i):
            t, jlo, jhi = chunks[i]
            nj = jhi - jlo
            zt = zpool.tile([128, 2 * R_PP * N_FEAT], FP16, tag="zt")
            if nj == R_PP:
                # whole tile: one load, one 14336 B run per partition
                nc.sync.dma_start(out=zt[:], in_=zv[t])
            else:
                nc.sync.dma_start(
                    out=zt[:, jlo * N_FEAT : jhi * N_FEAT],
                    in_=zv[t][:, jlo * N_FEAT : jhi * N_FEAT],
                )
                nc.sync.dma_start(
                    out=zt[:, KOFF + jlo * N_FEAT : KOFF + jhi * N_FEAT],
                    in_=zv[t][:, KOFF + jlo * N_FEAT : KOFF + jhi * N_FEAT],
                )
            return zt

        def emit_compute_store(i, zt):
            t, jlo, jhi = chunks[i]
            ot = opool.tile([128, R_PP * N_FEAT], FP16, tag="ot")
            for j in range(jlo, jhi):
                s = slice(j * N_FEAT, (j + 1) * N_FEAT)
                sk = slice(KOFF + j * N_FEAT, KOFF + (j + 1) * N_FEAT)
                c = t * R_PP + j
                nc.vector.scalar_tensor_tensor(
                    out=ot[:, s],
                    in0=zt[:, s],
                    scalar=nlb[:, c : c + 1],
                    in1=zt[:, sk],
                    op0=MULT,
                    op1=ADD,
                )
            nc.scalar.dma_start(
                out=ov[t][:, jlo * N_FEAT : jhi * N_FEAT],
                in_=ot[:, jlo * N_FEAT : jhi * N_FEAT],
            )

        # Loads run ZBUFS chunks ahead of compute; rings are single-purpose
        # so emission order only pipelines SBUF buffer reuse.
        zts = {}
        for i in range(min(ZBUFS, len(chunks))):
            zts[i] = emit_load(i)
        for i in range(len(chunks)):
            emit_compute_store(i, zts.pop(i))
            if i + ZBUFS < len(chunks):
                zts[i + ZBUFS] = emit_load(i + ZBUFS)

    nc.compile()
    return nc


def _get_nc():
    if "nc" not in _CACHE:
        _CACHE["nc"] = _build_nc()
    return _CACHE["nc"]


def _shuffle_lbda(lb_core):
    """[PAD_ROWS] -> [128, LB_COLS] with lb[p, t*R_PP+j] = lbda[t*TILE_ROWS + p*R_PP + j]."""
    return np.ascontiguousarray(
        lb_core.reshape(N_TILES, 128, R_PP)
        .transpose(1, 0, 2)
        .reshape(128, LB_COLS)
    )


def _make_in_maps(input_term, kipf_term, lbda):
    input_term = np.asarray(input_term, dtype=np.float32)
    kipf_term = np.asarray(kipf_term, dtype=np.float32)
    lbda = np.asarray(lbda, dtype=np.float32)
    in_maps = []
    for c in range(N_CORES):
        sl = slice(c * ROWS_PER_CORE, (c + 1) * ROWS_PER_CORE)
        xpadded = np.zeros((PAD_ROWS, N_FEAT), np.float16)
        xpadded[:ROWS_PER_CORE] = input_term[sl]
        kpadded = np.zeros((PAD_ROWS, N_FEAT), np.float16)
        kpadded[:ROWS_PER_CORE] = kipf_term[sl]
        # z: [t, p, h, j, c] — per (tile, partition) one contiguous run
        # holding that partition's input rows then its kipf rows.
        zc = np.empty((N_TILES, 128, 2, R_PP, N_FEAT), np.float16)
        zc[:, :, 0] = xpadded.reshape(N_TILES, 128, R_PP, N_FEAT)
        zc[:, :, 1] = kpadded.reshape(N_TILES, 128, R_PP, N_FEAT)
        lpadded = np.zeros((PAD_ROWS,), np.float32)
        lpadded[:ROWS_PER_CORE] = lbda[sl]
        in_maps.append(
            {
                "z": zc.reshape(2 * PAD_ROWS, N_FEAT),
                "lb": _shuffle_lbda(-lpadded),
            }
        )
    return in_maps


def kernel(input_term, kipf_term, lbda, spar=None, **_unused):
    from concourse.bass_utils import run_bass_kernel_spmd

    nc = _get_nc()
    in_maps = _make_in_maps(input_term, kipf_term, lbda)
    res = run_bass_kernel_spmd(nc, in_maps, list(range(N_CORES))).results
    return np.concatenate(
        [
            res[c]["o"][:ROWS_PER_CORE].astype(np.float32)
            for c in range(N_CORES)
        ],
        axis=0,
    )


# revision 5
# speedup vs baseline: 1.0682x; 1.0682x over previous
"""Trainium2 Bass kernel for nn_Damping_layer: out = kipf_term - lbda[:, None] * input_term.

Sharding: pure row-parallel over the n_nodes axis across 8 NeuronCores
(12500 rows per core), no cross-core communication. Each core's shard is
host-padded to 12544 rows so it divides into 7 uniform tiles of
[128 partitions x 14 rows/partition].

The kernel is HBM-bandwidth bound, so all DRAM I/O is fp16: the host
casts input_term/kipf_term to fp16 (norm relative error ~3e-4, far
inside the 2e-2 gate) and upcasts the fp16 output back to f32. This
halves DRAM traffic vs f32 (19.3 MB/core vs 38.5 MB/core).

input_term and kipf_term are interleaved on host into one DRAM tensor z
(per tile: 1792 input rows then 1792 kipf rows), so each tile needs a
single 1.75 MiB load. Loads and stores alternate between the two HWDGE
rings (SP and ACT) by tile parity, keeping both rings' byte demand equal
so the SDMA engines' per-queue round-robin matches the traffic mix.

The HWDGE sequencers spend ~7 us in framework preamble before their
first descriptor reaches the engines, but the SWDGE (gpsimd) queue
executes descriptors ~3 us earlier, so the first tile-0 sub-chunk load
rides SWDGE: the DMA engines start pulling HBM during the preamble
instead of idling.

lbda is negated and pre-shuffled on host into the matching
[partition, group] layout so the fused DVE op
    out = (input * (-lbda)) + kipf            (InstTensorScalarPtr)
consumes it directly as a per-partition scalar, one op per 128-row
group. The first/last tiles are emitted as small sub-chunks so the
pipeline ramps in and drains out quickly.
"""

import numpy as np

N_NODES = 100000
N_FEAT = 256
N_CORES = 8
ROWS_PER_CORE = N_NODES // N_CORES  # 12500

R_PP = 14                       # rows per partition in a tile
TILE_ROWS = 128 * R_PP          # 1792 rows per tile
N_TILES = 7                     # tiles per core
PAD_ROWS = N_TILES * TILE_ROWS  # 12544 rows per core after padding
LB_COLS = N_TILES * R_PP        # 98
N_BUFS = 6
W = 4                           # software-pipeline lookahead (chunks)

_CACHE = {}


def _build_nc():
    from contextlib import ExitStack

    import concourse.bacc as bacc
    import concourse.mybir as mybir
    import concourse.tile as tile

    FP32 = mybir.dt.float32
    FP16 = mybir.dt.float16
    nc = bacc.Bacc(
        "TRN2", target_bir_lowering=False, debug=False, num_devices=N_CORES
    )
    z = nc.dram_tensor(
        "z", [2 * PAD_ROWS, N_FEAT], FP16, kind="ExternalInput"
    ).ap()
    lb = nc.dram_tensor("lb", [128, LB_COLS], FP32, kind="ExternalInput").ap()
    o = nc.dram_tensor("o", [PAD_ROWS, N_FEAT], FP16, kind="ExternalOutput").ap()

    # z layout (host-built): [t, h, p, j, c] with h=0 input rows, h=1 kipf
    # rows; partition p holds R_PP*512B contiguous DRAM per (t, h).
    zv = z.rearrange(
        "(t h p j) c -> t p h (j c)", t=N_TILES, h=2, p=128, j=R_PP
    )
    ov = o.rearrange("(t p j) c -> t p (j c)", t=N_TILES, p=128, j=R_PP)

    MULT = mybir.AluOpType.mult
    ADD = mybir.AluOpType.add
    KOFF = R_PP * N_FEAT  # kipf half offset within a z tile

    with tile.TileContext(nc) as tc, ExitStack() as ctx:
        const = ctx.enter_context(tc.tile_pool(name="const", bufs=1))
        zpool = ctx.enter_context(tc.tile_pool(name="zp", bufs=N_BUFS))
        opool = ctx.enter_context(tc.tile_pool(name="op", bufs=N_BUFS))

        # Work list: first/last tiles in small sub-chunks so the pipeline
        # ramps in and drains out quickly; full tiles in between.
        chunks = [(0, 0, 4), (0, 4, 9), (0, 9, 14)]
        chunks += [(t, 0, R_PP) for t in range(1, N_TILES - 1)]
        chunks += [
            (N_TILES - 1, 0, 4),
            (N_TILES - 1, 4, 8),
            (N_TILES - 1, 8, 11),
            (N_TILES - 1, 11, 14),
        ]

        def ld_ring(i):
            # Chunk 0 rides SWDGE (emitted first) so the engines start
            # during the sequencer preamble. Tile 0's other sub-chunks on
            # SP and tile 1 (1.75 MiB) on ACT keep both rings' ramp load
            # equal; plain parity from there.
            if i == 0:
                return nc.gpsimd
            if i < 4:
                return nc.sync if i < 3 else nc.scalar
            return nc.sync if i % 2 == 0 else nc.scalar

        def st_ring(i):
            return nc.scalar if i % 2 == 0 else nc.sync

        def emit_load(i):
            t, jlo, jhi = chunks[i]
            nj = jhi - jlo
            zt = zpool.tile([128, 2 * R_PP * N_FEAT], FP16, tag="zt")
            eng = ld_ring(i)
            if nj == R_PP:
                # whole tile: one 1.75 MiB load covering both halves
                zt_hv = zt[:].rearrange("p (h f) -> p h f", h=2)
                eng.dma_start(out=zt_hv, in_=zv[t])
            else:
                eng.dma_start(
                    out=zt[:, jlo * N_FEAT : jhi * N_FEAT],
                    in_=zv[t][:, 0, jlo * N_FEAT : jhi * N_FEAT],
                )
                eng.dma_start(
                    out=zt[:, KOFF + jlo * N_FEAT : KOFF + jhi * N_FEAT],
                    in_=zv[t][:, 1, jlo * N_FEAT : jhi * N_FEAT],
                )
            return zt

        def emit_compute_store(i, zt):
            t, jlo, jhi = chunks[i]
            ot = opool.tile([128, R_PP * N_FEAT], FP16, tag="ot")
            for j in range(jlo, jhi):
                s = slice(j * N_FEAT, (j + 1) * N_FEAT)
                sk = slice(KOFF + j * N_FEAT, KOFF + (j + 1) * N_FEAT)
                c = t * R_PP + j
                nc.vector.scalar_tensor_tensor(
                    out=ot[:, s],
                    in0=zt[:, s],
                    scalar=nlb[:, c : c + 1],
                    in1=zt[:, sk],
                    op0=MULT,
                    op1=ADD,
                )
            st_ring(i).dma_start(
                out=ov[t][:, jlo * N_FEAT : jhi * N_FEAT],
                in_=ot[:, jlo * N_FEAT : jhi * N_FEAT],
            )

        # Chunk 0's load is the very first emitted instruction so SWDGE
        # starts filling the engines during the HWDGE preamble; nlb
        # follows it on the same SWDGE ring.
        zts = {}
        zts[0] = emit_load(0)
        nlb = const.tile([128, LB_COLS], FP32)
        nc.gpsimd.dma_start(out=nlb[:], in_=lb[:])

        # Software-pipelined emission: W chunk-loads run ahead so each
        # HWDGE ring's instruction stream starts with pure loads and no
        # store (gated on DVE) ever head-of-line-blocks the load front.
        for i in range(1, min(W, len(chunks))):
            zts[i] = emit_load(i)
        for i in range(len(chunks)):
            emit_compute_store(i, zts.pop(i))
            if i + W < len(chunks):
                zts[i + W] = emit_load(i + W)

    nc.compile()
    return nc


def _get_nc():
    if "nc" not in _CACHE:
        _CACHE["nc"] = _build_nc()
    return _CACHE["nc"]


def _shuffle_lbda(lb_core):
    """[PAD_ROWS] -> [128, LB_COLS] with lb[p, t*R_PP+j] = lbda[t*TILE_ROWS + p*R_PP + j]."""
    return np.ascontiguousarray(
        lb_core.reshape(N_TILES, 128, R_PP)
        .transpose(1, 0, 2)
        .reshape(128, LB_COLS)
    )


def _make_in_maps(input_term, kipf_term, lbda):
    input_term = np.asarray(input_term, dtype=np.float32)
    kipf_term = np.asarray(kipf_term, dtype=np.float32)
    lbda = np.asarray(lbda, dtype=np.float32)
    in_maps = []
    for c in range(N_CORES):
        sl = slice(c * ROWS_PER_CORE, (c + 1) * ROWS_PER_CORE)
        xpadded = np.zeros((PAD_ROWS, N_FEAT), np.float16)
        xpadded[:ROWS_PER_CORE] = input_term[sl]
        kpadded = np.zeros((PAD_ROWS, N_FEAT), np.float16)
        kpadded[:ROWS_PER_CORE] = kipf_term[sl]
        # z: per tile, TILE_ROWS input rows then TILE_ROWS kipf rows
        zc = np.empty((N_TILES, 2, TILE_ROWS, N_FEAT), np.float16)
        zc[:, 0] = xpadded.reshape(N_TILES, TILE_ROWS, N_FEAT)
        zc[:, 1] = kpadded.reshape(N_TILES, TILE_ROWS, N_FEAT)
        lpadded = np.zeros((PAD_ROWS,), np.float32)
        lpadded[:ROWS_PER_CORE] = lbda[sl]
        in_maps.append(
            {
                "z": zc.reshape(2 * PAD_ROWS, N_FEAT),
                "lb": _shuffle_lbda(-lpadded),
            }
        )
    return in_maps


def kernel(input_term, kipf_term, lbda, spar=None, **_unused):
    from concourse.bass_utils import run_bass_kernel_spmd

    nc = _get_nc()
    in_maps = _make_in_maps(input_term, kipf_term, lbda)
    res = run_bass_kernel_spmd(nc, in_maps, list(range(N_CORES))).results
    return np.concatenate(
        [
            res[c]["o"][:ROWS_PER_CORE].astype(np.float32)
            for c in range(N_CORES)
        ],
        axis=0,
    )


# revision 6
# speedup vs baseline: 1.1306x; 1.0584x over previous
"""Trainium2 Bass kernel for nn_Damping_layer: out = kipf_term - lbda[:, None] * input_term.

Sharding: pure row-parallel over the n_nodes axis across 8 NeuronCores
(12500 rows per core), no cross-core communication. Each core's shard is
host-padded to 12544 rows so it divides into 7 uniform tiles of
[128 partitions x 14 rows/partition].

The kernel is HBM-bandwidth bound, so all DRAM I/O is fp16: the host
casts input_term/kipf_term to fp16 (norm relative error ~3e-4, far
inside the 2e-2 gate) and upcasts the fp16 output back to f32. This
halves DRAM traffic vs f32 (19.3 MB/core vs 38.5 MB/core).

input_term and kipf_term are interleaved on host into one DRAM tensor z
(per tile: 1792 input rows then 1792 kipf rows), so each tile needs a
single 1.75 MiB load. Loads and stores alternate between the two HWDGE
rings (SP and ACT) by tile parity, keeping both rings' byte demand equal
so the SDMA engines' per-queue round-robin matches the traffic mix.

lbda is pre-shuffled on host into the matching [partition, group] layout
so the fused DVE op
    out = (input * (-lbda)) + kipf            (InstTensorScalarPtr)
consumes it as a per-partition scalar, one op per 128-row group. The
first/last tiles are emitted as small sub-chunks so the pipeline ramps
in and drains out quickly.
"""

import numpy as np

N_NODES = 100000
N_FEAT = 256
N_CORES = 8
ROWS_PER_CORE = N_NODES // N_CORES  # 12500

R_PP = 14                       # rows per partition in a tile
TILE_ROWS = 128 * R_PP          # 1792 rows per tile
N_TILES = 7                     # tiles per core
PAD_ROWS = N_TILES * TILE_ROWS  # 12544 rows per core after padding
LB_COLS = N_TILES * R_PP        # 98
N_BUFS = 6

_CACHE = {}


def _build_nc():
    from contextlib import ExitStack

    import concourse.bacc as bacc
    import concourse.mybir as mybir
    import concourse.tile as tile

    FP32 = mybir.dt.float32
    FP16 = mybir.dt.float16
    nc = bacc.Bacc(
        "TRN2", target_bir_lowering=False, debug=False, num_devices=N_CORES
    )
    z = nc.dram_tensor(
        "z", [2 * PAD_ROWS, N_FEAT], FP16, kind="ExternalInput"
    ).ap()
    lb = nc.dram_tensor("lb", [128, LB_COLS], FP32, kind="ExternalInput").ap()
    o = nc.dram_tensor("o", [PAD_ROWS, N_FEAT], FP16, kind="ExternalOutput").ap()

    # z layout (host-built): [t, h, p, j, c] with h=0 input rows, h=1 kipf
    # rows; partition p holds R_PP*512B contiguous DRAM per (t, h).
    zv = z.rearrange(
        "(t h p j) c -> t p h (j c)", t=N_TILES, h=2, p=128, j=R_PP
    )
    ov = o.rearrange("(t p j) c -> t p (j c)", t=N_TILES, p=128, j=R_PP)

    MULT = mybir.AluOpType.mult
    ADD = mybir.AluOpType.add
    KOFF = R_PP * N_FEAT  # kipf half offset within a z tile

    with tile.TileContext(nc) as tc, ExitStack() as ctx:
        const = ctx.enter_context(tc.tile_pool(name="const", bufs=1))
        zpool = ctx.enter_context(tc.tile_pool(name="zp", bufs=N_BUFS))
        opool = ctx.enter_context(tc.tile_pool(name="op", bufs=N_BUFS))

        # lbt rides SWDGE (gpsimd), keeping both HWDGE rings' heads free
        # for the first data loads.
        lbt = const.tile([128, LB_COLS], FP32)
        nc.gpsimd.dma_start(out=lbt[:], in_=lb[:])
        nlb = const.tile([128, LB_COLS], FP32)
        nc.vector.tensor_scalar_mul(nlb[:], lbt[:], -1.0)

        # Work list: first/last tiles in small sub-chunks so the pipeline
        # ramps in and drains out quickly; full tiles in between.
        chunks = [(0, 0, 4), (0, 4, 9), (0, 9, 14)]
        chunks += [(t, 0, R_PP) for t in range(1, N_TILES - 1)]
        chunks += [
            (N_TILES - 1, 0, 4),
            (N_TILES - 1, 4, 8),
            (N_TILES - 1, 8, 11),
            (N_TILES - 1, 11, 14),
        ]

        def ld_ring(i):
            # Prefix: tile 0's three sub-chunks on SP and tile 1 (1.75 MiB)
            # on ACT, so both rings hold equal load bytes during the ramp;
            # plain parity from there (full tiles are all equal-sized).
            if i < 4:
                return nc.sync if i < 3 else nc.scalar
            return nc.sync if i % 2 == 0 else nc.scalar

        def st_ring(i):
            return nc.scalar if i % 2 == 0 else nc.sync

        def emit_load(i):
            t, jlo, jhi = chunks[i]
            nj = jhi - jlo
            zt = zpool.tile([128, 2 * R_PP * N_FEAT], FP16, tag="zt")
            eng = ld_ring(i)
            if nj == R_PP:
                # whole tile: one 1.75 MiB load covering both halves
                zt_hv = zt[:].rearrange("p (h f) -> p h f", h=2)
                eng.dma_start(out=zt_hv, in_=zv[t])
            else:
                eng.dma_start(
                    out=zt[:, jlo * N_FEAT : jhi * N_FEAT],
                    in_=zv[t][:, 0, jlo * N_FEAT : jhi * N_FEAT],
                )
                eng.dma_start(
                    out=zt[:, KOFF + jlo * N_FEAT : KOFF + jhi * N_FEAT],
                    in_=zv[t][:, 1, jlo * N_FEAT : jhi * N_FEAT],
                )
            return zt

        def emit_compute_store(i, zt):
            t, jlo, jhi = chunks[i]
            ot = opool.tile([128, R_PP * N_FEAT], FP16, tag="ot")
            for j in range(jlo, jhi):
                s = slice(j * N_FEAT, (j + 1) * N_FEAT)
                sk = slice(KOFF + j * N_FEAT, KOFF + (j + 1) * N_FEAT)
                c = t * R_PP + j
                nc.vector.scalar_tensor_tensor(
                    out=ot[:, s],
                    in0=zt[:, s],
                    scalar=nlb[:, c : c + 1],
                    in1=zt[:, sk],
                    op0=MULT,
                    op1=ADD,
                )
            st_ring(i).dma_start(
                out=ov[t][:, jlo * N_FEAT : jhi * N_FEAT],
                in_=ot[:, jlo * N_FEAT : jhi * N_FEAT],
            )

        # Software-pipelined emission: W chunk-loads run ahead so each
        # HWDGE ring's instruction stream starts with pure loads and no
        # store (gated on DVE) ever head-of-line-blocks the load front.
        W = 4
        zts = {}
        for i in range(min(W, len(chunks))):
            zts[i] = emit_load(i)
        for i in range(len(chunks)):
            emit_compute_store(i, zts.pop(i))
            if i + W < len(chunks):
                zts[i + W] = emit_load(i + W)

    nc.compile()
    return nc


def _get_nc():
    if "nc" not in _CACHE:
        _CACHE["nc"] = _build_nc()
    return _CACHE["nc"]


def _shuffle_lbda(lb_core):
    """[PAD_ROWS] -> [128, LB_COLS] with lb[p, t*R_PP+j] = lbda[t*TILE_ROWS + p*R_PP + j]."""
    return np.ascontiguousarray(
        lb_core.reshape(N_TILES, 128, R_PP)
        .transpose(1, 0, 2)
        .reshape(128, LB_COLS)
    )


def _make_in_maps(input_term, kipf_term, lbda):
    input_term = np.asarray(input_term, dtype=np.float32)
    kipf_term = np.asarray(kipf_term, dtype=np.float32)
    lbda = np.asarray(lbda, dtype=np.float32)
    in_maps = []
    for c in range(N_CORES):
        sl = slice(c * ROWS_PER_CORE, (c + 1) * ROWS_PER_CORE)
        xpadded = np.zeros((PAD_ROWS, N_FEAT), np.float16)
        xpadded[:ROWS_PER_CORE] = input_term[sl]
        kpadded = np.zeros((PAD_ROWS, N_FEAT), np.float16)
        kpadded[:ROWS_PER_CORE] = kipf_term[sl]
        # z: per tile, TILE_ROWS input rows then TILE_ROWS kipf rows
        zc = np.empty((N_TILES, 2, TILE_ROWS, N_FEAT), np.float16)
        zc[:, 0] = xpadded.reshape(N_TILES, TILE_ROWS, N_FEAT)
        zc[:, 1] = kpadded.reshape(N_TILES, TILE_ROWS, N_FEAT)
        lpadded = np.zeros((PAD_ROWS,), np.float32)
        lpadded[:ROWS_PER_CORE] = lbda[sl]
        in_maps.append(
            {
                "z": zc.reshape(2 * PAD_ROWS, N_FEAT),
                "lb": _shuffle_lbda(lpadded),
            }
        )
    return in_maps


def kernel(input_term, kipf_term, lbda, spar=None, **_unused):
    from concourse.bass_utils import run_bass_kernel_spmd

    nc = _get_nc()
    in_maps = _make_in_maps(input_term, kipf_term, lbda)
    res = run_bass_kernel_spmd(nc, in_maps, list(range(N_CORES))).results
    return np.concatenate(
        [
            res[c]["o"][:ROWS_PER_CORE].astype(np.float32)
            for c in range(N_CORES)
        ],
        axis=0,
    )
